# revision 1
# baseline (speedup 1.0000x reference)
# Trainium2 Bass kernel for nn_MoD_8022998909591 (mixture-of-depths routing +
# transformer block). Self-contained: accepts FULL inputs, returns FULL output.
#
# Sharding: data-parallel over batch (4 rows -> 4 core pairs), tensor-parallel
# within each pair (16 heads -> 8+8, FFN 8192 -> 4096+4096; wq/wk/wv col-shard,
# wo/w2 row-shard with a pairwise bf16 AllReduce after each). Router, top-k,
# sort, gather/scatter run per-core on the core's batch row.
#
# Per-core layout strategy: activations flow feature-major [D, T] through the
# matmuls (weights in natural [din, dout] order serve directly as lhsT), while
# the residual stream stays token-major [T, D]; sublayer outputs are transposed
# back token-major with PE-transposes (bf16, 1 cyc/row).
#
# Top-k (exact, data-parallel): 128-way threshold bisection on the router
# logits. Each pass evaluates count(logit >= t_i) for 128 candidate thresholds
# at once (one fused DVE compare+accumulate over a [128, 4096] broadcast of the
# logits), then narrows [lo, hi) by 128x. 7 passes isolate the exact 1024th
# value. Selected indices are compacted in ascending order via a prefix-scan of
# the selection mask + indirect-DMA scatter.

import contextlib

import numpy as np

import concourse.bacc as bacc
import concourse.bass as bass
import concourse.bass_isa as bass_isa
import concourse.mybir as mybir
import concourse.tile as tile
from concourse.bass_utils import run_bass_kernel_spmd
from concourse.masks import make_identity

B, S, D = 4, 4096, 2048
H, HD = 16, 128
FFN = 8192
K = 1024                      # top-k tokens per row
EPS = 1e-6
P = 128
NCORE = 8
HALF = S // 2                 # output region rows per core
DH = D // 2                   # per-core head width (8 heads * 128)
FH = FFN // 2                 # per-core FFN width
HEADS = H // 2                # heads per core
NT = K // P                   # 8 token tiles of the selected set
ND = D // P                   # 16 feature tiles
NF = FH // P                  # 32 FFN chunks per core
GROUPS = [[0, 1], [2, 3], [4, 5], [6, 7]]
BISECT_PASSES = 7

dt = mybir.dt
f32, bf16, i32 = dt.float32, dt.bfloat16, dt.int32
OP = mybir.AluOpType
ACT = mybir.ActivationFunctionType


def _build_program(solo=False, upto=99):
    nc = bacc.Bacc("TRN2", target_bir_lowering=False, debug=False,
                   num_devices=NCORE)

    io = dict(
        xr=nc.dram_tensor("xr", [S, D], f32, kind="ExternalInput"),
        xregion=nc.dram_tensor("xregion", [HALF, D], f32, kind="ExternalInput"),
        selbase=nc.dram_tensor("selbase", [P, 1], f32, kind="ExternalInput"),
        wrb=nc.dram_tensor("wrb", [P, D], f32, kind="ExternalInput"),
        cosT=nc.dram_tensor("cosT", [HD // 2, K], f32, kind="ExternalInput"),
        sinT=nc.dram_tensor("sinT", [HD // 2, K], f32, kind="ExternalInput"),
        iotaP=nc.dram_tensor("iotaP", [P, 1], f32, kind="ExternalInput"),
        iotaP1=nc.dram_tensor("iotaP1", [P, 1], f32, kind="ExternalInput"),
        itok=nc.dram_tensor("itok", [P, 32], i32, kind="ExternalInput"),
        wqt=nc.dram_tensor("wqt", [HEADS, D, P], bf16, kind="ExternalInput"),
        wkt=nc.dram_tensor("wkt", [HEADS, D, P], bf16, kind="ExternalInput"),
        wvt=nc.dram_tensor("wvt", [D, DH], bf16, kind="ExternalInput"),
        wot=nc.dram_tensor("wot", [ND, DH, P], bf16, kind="ExternalInput"),
        w1t=nc.dram_tensor("w1t", [NF, D, P], bf16, kind="ExternalInput"),
        w3t=nc.dram_tensor("w3t", [NF, D, P], bf16, kind="ExternalInput"),
        w2t=nc.dram_tensor("w2t", [ND, FH, P], bf16, kind="ExternalInput"),
        outr=nc.dram_tensor("outr", [HALF, D], f32, kind="ExternalOutput"),
    )

    with tile.TileContext(nc) as tc:
        _emit(nc, tc, io, solo=solo, upto=upto)
    nc.compile()
    return nc


def _rmsnorm_tile(nc, pool, xt, out_bf, tag):
    """out_bf = xt * rsqrt(mean(xt^2) + eps), token-major [P, D] tile."""
    ssq = pool.tile([P, 1], f32, tag=tag + "ssq")
    sdum = pool.tile([P, 1], f32, tag=tag + "sdum")
    nc.scalar.activation(out=sdum[:].to_broadcast([P, D]), in_=xt[:],
                         func=ACT.Square, accum_out=ssq[:])
    nc.vector.tensor_scalar(out=ssq[:], in0=ssq[:], scalar1=1.0 / D,
                            scalar2=EPS, op0=OP.mult, op1=OP.add)
    nc.scalar.activation(out=ssq[:], in_=ssq[:], func=ACT.Sqrt)
    rs = pool.tile([P, 1], f32, tag=tag + "rs")
    nc.vector.reciprocal(out=rs[:], in_=ssq[:])
    nc.vector.tensor_scalar(out=out_bf[:], in0=xt[:], scalar1=rs[:, :1],
                            scalar2=None, op0=OP.mult)


def _emit(nc, tc, io, solo=False, upto=99):
    xr, xregion, selbase = io["xr"], io["xregion"], io["selbase"]
    wrb, cosT, sinT = io["wrb"], io["cosT"], io["sinT"]
    iotaP, iotaP1, itok = io["iotaP"], io["iotaP1"], io["itok"]
    wqt, wkt, wvt, wot = io["wqt"], io["wkt"], io["wvt"], io["wot"]
    w1t, w3t, w2t, outr = io["w1t"], io["w3t"], io["w2t"], io["outr"]

    ctx = contextlib.ExitStack()
    with ctx:
        dram = ctx.enter_context(tc.tile_pool(name="dram", bufs=1, space="DRAM"))
        keep = ctx.enter_context(tc.tile_pool(name="keep", bufs=1))

        # long-lived small tiles
        SI = keep.tile([P, NT], i32)        # selected indices, sel order
        SIDX = keep.tile([P, NT], i32)      # region-shifted scatter indices
        RW = keep.tile([P, NT], f32)        # softmax router weights, sel order
        ident_b = keep.tile([P, P], bf16)
        ones_b = keep.tile([P, 1], bf16)
        cos_t = keep.tile([HD // 2, K], f32)
        sin_t = keep.tile([HD // 2, K], f32)
        ident_f = keep.tile([P, P], f32)
        make_identity(nc, ident_f[:])
        nc.vector.tensor_copy(out=ident_b[:], in_=ident_f[:])
        nc.vector.memset(ones_b[:], 1.0)
        nc.sync.dma_start(out=cos_t[:], in_=cosT[:])
        nc.sync.dma_start(out=sin_t[:], in_=sinT[:])

        # internal DRAM scratch
        ld = dram.tile([S, 1], f32)         # router logits, token order
        seld = dram.tile([K, 1], i32)       # selected indices, ascending
        atd = dram.tile([K, D], bf16)       # attn partial, token-major
        atrd = dram.tile([K, D], bf16)      # attn reduced
        ftd = dram.tile([K, D], bf16)       # ffn partial, token-major
        ftrd = dram.tile([K, D], bf16)      # ffn reduced
        x1d = dram.tile([K, D], f32)        # post-attn residual

        # output passthrough copy; scatters later overwrite selected rows.
        # Chunked so the copies spread across DMA queues and finish early --
        # every tail scatter WAW-waits on this tensor.
        for cc in range(16):
            nc.sync.dma_start(out=outr[cc * P:(cc + 1) * P, :],
                              in_=xregion[cc * P:(cc + 1) * P, :])

        # ================= phase R: router + top-k =================
        with tc.tile_pool(name="route", bufs=1) as rp, \
             tc.tile_pool(name="xin", bufs=3) as xp, \
             tc.tile_pool(name="rps", bufs=2, space="PSUM") as rps:
            LG = rp.tile([P, 32], f32)      # logits, token t = c*128+p
            WRB = rp.tile([P, D], f32)
            nc.sync.dma_start(out=WRB[:], in_=wrb[:])
            rdum = rp.tile([P, 1], f32)
            for c in range(32):
                xt = xp.tile([P, D], f32, tag="xt")
                nc.sync.dma_start(out=xt[:], in_=xr[c * P:(c + 1) * P, :])
                nc.vector.affine_mul_reduce(
                    out=rdum[:].to_broadcast([P, D]), accum_out=LG[:, c:c + 1],
                    in0=xt[:], in1=WRB[:], scale=1.0, bias=0.0)
            # transpose LG on PE so the DRAM write is contiguous in token
            # order (a direct [p,c] -> (c p) DMA would issue 4096 scattered
            # 4-byte writes on the serial critical path)
            lgt_ps = rps.tile([32, P], f32, space="PSUM")
            nc.tensor.matmul(lgt_ps[:], lhsT=LG[:], rhs=ident_f[:],
                             start=True, stop=True)
            LGT = rp.tile([32, P], f32)
            nc.vector.tensor_copy(out=LGT[:], in_=lgt_ps[:])
            nc.sync.dma_start(out=ld[:, 0].rearrange("(c p) -> c p", p=P),
                              in_=LGT[:])
            LB = rp.tile([P, S], f32)       # logits broadcast across partitions
            LF = rp.tile([P, 32], f32)      # token t = p*32 + c
            nc.sync.dma_start(out=LB[:],
                              in_=ld[:, 0][None, :].to_broadcast([P, S]))
            nc.sync.dma_start(out=LF[:],
                              in_=ld[:, 0].rearrange("(p c) -> p c", c=32))

            # initial [lo, hi): lo = min(logits), hi = max(logits)
            lo = rp.tile([P, 1], f32)
            hi = rp.tile([P, 1], f32)
            stp = rp.tile([P, 1], f32)
            T = rp.tile([P, 1], f32)
            CNT = rp.tile([P, 1], f32)
            ok = rp.tile([P, 1], f32)
            sv = rp.tile([P, 1], f32)
            ist = rp.tile([P, 1], f32)
            IOTA = rp.tile([P, 1], f32)
            IOTA1 = rp.tile([P, 1], f32)
            ones_f = rp.tile([P, 1], f32)
            neg1 = rp.tile([P, 1], f32)
            scr = rp.tile([P, 32], f32)
            cdum = rp.tile([P, 1], f32)
            nc.sync.dma_start(out=IOTA[:], in_=iotaP[:])
            nc.sync.dma_start(out=IOTA1[:], in_=iotaP1[:])
            nc.vector.memset(ones_f[:], 1.0)
            nc.vector.memset(neg1[:], -1.0)
            nc.vector.tensor_scalar(out=scr[:], in0=LF[:], scalar1=-1.0,
                                    scalar2=None, op0=OP.mult)
            nc.vector.tensor_reduce(out=lo[:], in_=scr[:],
                                    axis=mybir.AxisListType.X, op=OP.max)
            nc.gpsimd.partition_all_reduce(lo[:], lo[:], P, bass_isa.ReduceOp.max)
            nc.vector.tensor_scalar(out=lo[:], in0=lo[:], scalar1=-1.0,
                                    scalar2=None, op0=OP.mult)
            nc.vector.tensor_reduce(out=hi[:], in_=LF[:],
                                    axis=mybir.AxisListType.X, op=OP.max)
            nc.gpsimd.partition_all_reduce(hi[:], hi[:], P, bass_isa.ReduceOp.max)

            for _ in range(BISECT_PASSES):
                # step = (hi - lo) / 128 ; T_p = lo + p * step
                nc.vector.tensor_sub(out=stp[:], in0=hi[:], in1=lo[:])
                nc.vector.tensor_scalar(out=stp[:], in0=stp[:],
                                        scalar1=1.0 / 128, scalar2=None,
                                        op0=OP.mult)
                nc.vector.scalar_tensor_tensor(
                    out=T[:], in0=IOTA[:], scalar=stp[:, :1], in1=lo[:],
                    op0=OP.mult, op1=OP.add)
                # count(logit >= T_p) per partition p
                nc.vector.scalar_tensor_tensor(
                    out=cdum[:].to_broadcast([P, S]), in0=LB[:],
                    scalar=T[:, :1], in1=ones_f[:].to_broadcast([P, S]),
                    op0=OP.is_ge, op1=OP.mult, accum_out=CNT[:])
                # i* = max{i : count_i >= K}; sv = i*+1 (count_0 >= K always)
                nc.vector.tensor_scalar(out=ok[:], in0=CNT[:], scalar1=float(K),
                                        scalar2=None, op0=OP.is_ge)
                nc.vector.scalar_tensor_tensor(
                    out=sv[:], in0=ok[:], scalar=IOTA1[:, :1], in1=neg1[:],
                    op0=OP.mult, op1=OP.add)
                nc.gpsimd.partition_all_reduce(sv[:], sv[:], P,
                                               bass_isa.ReduceOp.max)
                # sv = i* = largest grid index with count >= K
                nc.vector.scalar_tensor_tensor(
                    out=lo[:], in0=sv[:], scalar=stp[:, :1], in1=lo[:],
                    op0=OP.mult, op1=OP.add)
                nc.vector.tensor_add(out=hi[:], in0=lo[:], in1=stp[:])

            # selection mask over LF (token t = p*32+c), compact via scan
            mask = rp.tile([P, 32], f32)
            SC = rp.tile([P, 32], f32)
            nc.vector.tensor_scalar(out=mask[:], in0=LF[:], scalar1=lo[:, :1],
                                    scalar2=None, op0=OP.is_ge)
            nc.vector.tensor_tensor_scan(
                out=SC[:], data0=mask[:], data1=mask[:], initial=0.0,
                op0=OP.add, op1=OP.bypass)
            # cross-partition exclusive prefix of per-partition totals
            totT_ps = rps.tile([1, P], f32, space="PSUM")
            nc.tensor.matmul(totT_ps[:], lhsT=SC[:, 31:32], rhs=ident_f[:],
                             start=True, stop=True)
            totT = rp.tile([1, P], f32)
            nc.vector.tensor_copy(out=totT[:], in_=totT_ps[:])
            scT = rp.tile([1, P], f32)
            nc.vector.tensor_tensor_scan(
                out=scT[:], data0=totT[:], data1=totT[:], initial=0.0,
                op0=OP.add, op1=OP.bypass)
            nc.vector.tensor_sub(out=scT[:], in0=scT[:], in1=totT[:])
            base_ps = rps.tile([P, 1], f32, space="PSUM")
            one_f = rp.tile([1, 1], f32)
            nc.vector.memset(one_f[:], 1.0)
            nc.tensor.matmul(base_ps[:], lhsT=scT[:], rhs=one_f[:],
                             start=True, stop=True)
            base = rp.tile([P, 1], f32)
            nc.vector.tensor_copy(out=base[:], in_=base_ps[:])
            # pos = SC - mask + base, unselected -> +1e9 (OOB, skipped)
            pos = rp.tile([P, 32], f32)
            nc.vector.tensor_sub(out=pos[:], in0=SC[:], in1=mask[:])
            nc.vector.tensor_scalar(out=pos[:], in0=pos[:], scalar1=base[:, :1],
                                    scalar2=None, op0=OP.add)
            q1 = rp.tile([P, 32], f32)
            nc.vector.tensor_scalar(out=q1[:], in0=mask[:], scalar1=-1e9,
                                    scalar2=1e9, op0=OP.mult, op1=OP.add)
            nc.vector.tensor_add(out=pos[:], in0=pos[:], in1=q1[:])
            posi = rp.tile([P, 32], i32)
            nc.vector.tensor_copy(out=posi[:], in_=pos[:])
            ITOK = rp.tile([P, 32], i32)
            nc.sync.dma_start(out=ITOK[:], in_=itok[:])
            zsel = rp.tile([P, NT], i32)
            nc.vector.memset(zsel[:], 0)
            nc.sync.dma_start(out=seld[:, 0].rearrange("(j p) -> p j", p=P),
                              in_=zsel[:])
            for c in range(32):
                nc.gpsimd.indirect_dma_start(
                    out=seld[:], out_offset=bass.IndirectOffsetOnAxis(
                        ap=posi[:, c:c + 1], axis=0),
                    in_=ITOK[:, c:c + 1], in_offset=None,
                    bounds_check=K - 1, oob_is_err=False)
            # read back selected indices (ascending); region-shifted variant
            nc.sync.dma_start(
                out=SI[:], in_=seld[:, 0].rearrange("(j p) -> p j", p=P))
            SB = rp.tile([P, 1], f32)
            nc.sync.dma_start(out=SB[:], in_=selbase[:])
            sif = rp.tile([P, NT], f32)
            nc.vector.tensor_copy(out=sif[:], in_=SI[:])
            nc.vector.tensor_scalar(out=sif[:], in0=sif[:], scalar1=SB[:, :1],
                                    scalar2=None, op0=OP.subtract)
            sneg = rp.tile([P, NT], f32)
            nc.vector.tensor_scalar(out=sneg[:], in0=sif[:], scalar1=0.0,
                                    scalar2=1e9, op0=OP.is_lt, op1=OP.mult)
            nc.vector.tensor_add(out=sif[:], in0=sif[:], in1=sneg[:])
            nc.vector.tensor_copy(out=SIDX[:], in_=sif[:])
            # router softmax over selected logits (sel order)
            WS = rp.tile([P, NT], f32)
            for j in range(NT):
                nc.gpsimd.indirect_dma_start(
                    out=WS[:, j:j + 1], out_offset=None, in_=ld[:],
                    in_offset=bass.IndirectOffsetOnAxis(ap=SI[:, j:j + 1],
                                                        axis=0))
            mx = rp.tile([P, 1], f32)
            nc.vector.tensor_reduce(out=mx[:], in_=WS[:],
                                    axis=mybir.AxisListType.X, op=OP.max)
            nc.gpsimd.partition_all_reduce(mx[:], mx[:], P, bass_isa.ReduceOp.max)
            nmx = rp.tile([P, 1], f32)
            nc.vector.tensor_scalar(out=nmx[:], in0=mx[:], scalar1=-1.0,
                                    scalar2=None, op0=OP.mult)
            zsm = rp.tile([P, 1], f32)
            nc.scalar.activation(out=RW[:], in_=WS[:], func=ACT.Exp,
                                 bias=nmx[:, :1], accum_out=zsm[:])
            nc.gpsimd.partition_all_reduce(zsm[:], zsm[:], P,
                                           bass_isa.ReduceOp.add)
            zrc = rp.tile([P, 1], f32)
            nc.vector.reciprocal(out=zrc[:], in_=zsm[:])
            nc.vector.tensor_scalar(out=RW[:], in0=RW[:], scalar1=zrc[:, :1],
                                    scalar2=None, op0=OP.mult)

        if upto <= 1:
            return
        # ============ phases G/A: gather, norm, QKV, attention, wo ============
        with tc.tile_pool(name="hfp", bufs=1) as hfp:
            HF = hfp.tile([P, ND, K], bf16)     # h feature-major [din, t]
            with tc.tile_pool(name="gat", bufs=3) as gp, \
                 tc.tile_pool(name="gps", bufs=4, space="PSUM") as gps:
                for j in range(NT):
                    fxt = gp.tile([P, D], f32, tag="fx")
                    nc.gpsimd.indirect_dma_start(
                        out=fxt[:], out_offset=None, in_=xr[:],
                        in_offset=bass.IndirectOffsetOnAxis(ap=SI[:, j:j + 1],
                                                            axis=0))
                    ht = gp.tile([P, D], bf16, tag="ht")
                    _rmsnorm_tile(nc, gp, fxt, ht, "g")
                    for i in range(ND):
                        tp = gps.tile([P, P], bf16, space="PSUM", tag="tp")
                        nc.tensor.transpose(out=tp[:],
                                            in_=ht[:, i * P:(i + 1) * P],
                                            identity=ident_b[:])
                        nc.scalar.copy(out=HF[:, i, j * P:(j + 1) * P],
                                       in_=tp[:])

            if upto <= 2:
                return
            with tc.tile_pool(name="qkp", bufs=1) as qkp:
                QF = qkp.tile([P, HEADS, K], bf16)
                KF = qkp.tile([P, HEADS, K], bf16)
                VT = qkp.tile([P, NT, DH], bf16)
                ATT = qkp.tile([P, HEADS, K], bf16)
                with tc.tile_pool(name="wld", bufs=3) as wp, \
                     tc.tile_pool(name="rope", bufs=2) as rop, \
                     tc.tile_pool(name="aps", bufs=2, space="PSUM") as aps:
                    for h in range(HEADS):
                        for dst, wsrc in ((QF, wqt), (KF, wkt)):
                            wqs = wp.tile([P, ND, P], bf16, tag="wqs")
                            nc.sync.dma_start(
                                out=wqs[:],
                                in_=wsrc[h].rearrange("(i p) n -> p i n", p=P))
                            q_ps = aps.tile([P, K], f32, space="PSUM", tag="qps")
                            for i in range(ND):
                                for ch in range(2):
                                    cs = slice(ch * 512, (ch + 1) * 512)
                                    nc.tensor.matmul(
                                        q_ps[:, cs], lhsT=wqs[:, i, :],
                                        rhs=HF[:, i, cs],
                                        start=(i == 0), stop=(i == ND - 1))
                            # rope: rows 0:64 = x1 half, 64:128 = x2 half
                            ta = rop.tile([64, K], f32, tag="ta")
                            tb = rop.tile([64, K], f32, tag="tb")
                            nc.vector.tensor_mul(out=ta[:], in0=q_ps[0:64, :],
                                                 in1=cos_t[:])
                            nc.vector.tensor_mul(out=tb[:], in0=q_ps[64:128, :],
                                                 in1=sin_t[:])
                            nc.vector.tensor_sub(out=dst[0:64, h, :],
                                                 in0=ta[:], in1=tb[:])
                            nc.vector.tensor_mul(out=ta[:], in0=q_ps[0:64, :],
                                                 in1=sin_t[:])
                            nc.vector.tensor_mul(out=tb[:], in0=q_ps[64:128, :],
                                                 in1=cos_t[:])
                            nc.vector.tensor_add(out=dst[64:128, h, :],
                                                 in0=ta[:], in1=tb[:])
                    with tc.tile_pool(name="wvp", bufs=1) as wvp:
                        WV = wvp.tile([P, ND, DH], bf16)
                        nc.sync.dma_start(
                            out=WV[:],
                            in_=wvt[:].rearrange("(i p) n -> p i n", p=P))
                        for j in range(NT):
                            v_ps = aps.tile([P, DH], f32, space="PSUM",
                                            tag="vps")
                            for i in range(ND):
                                for ch in range(2):
                                    cs = slice(ch * 512, (ch + 1) * 512)
                                    nc.tensor.matmul(
                                        v_ps[:, cs],
                                        lhsT=HF[:, i, j * P:(j + 1) * P],
                                        rhs=WV[:, i, cs],
                                        start=(i == 0), stop=(i == ND - 1))
                            nc.scalar.copy(out=VT[:, j, :], in_=v_ps[:])

                if upto <= 3:
                    return
                # attention: scores^T tile-streamed, softmax without max-sub
                # (|scores/sqrt(hd)| <~ 5), Z via ones-matmul, deferred 1/Z
                with tc.tile_pool(name="att", bufs=3) as ap, \
                     tc.tile_pool(name="sps", bufs=2, space="PSUM") as sps, \
                     tc.tile_pool(name="ops", bufs=1, space="PSUM") as ops:
                    for h in range(HEADS):
                        o_ps = ops.tile([P, K], f32, space="PSUM", tag="ops")
                        z_ps = ops.tile([1, K], f32, space="PSUM", tag="zps")
                        for j in range(NT):
                            s_ps = sps.tile([P, K], f32, space="PSUM", tag="sps")
                            for ch in range(2):
                                cs = slice(ch * 512, (ch + 1) * 512)
                                nc.tensor.matmul(
                                    s_ps[:, cs],
                                    lhsT=KF[:, h, j * P:(j + 1) * P],
                                    rhs=QF[:, h, cs], start=True, stop=True)
                            ex = ap.tile([P, K], bf16, tag="ex")
                            nc.scalar.activation(
                                out=ex[:], in_=s_ps[:], func=ACT.Exp,
                                scale=float(1.0 / np.sqrt(HD)))
                            for ch in range(2):
                                cs = slice(ch * 512, (ch + 1) * 512)
                                nc.tensor.matmul(
                                    z_ps[:, cs], lhsT=ones_b[:], rhs=ex[:, cs],
                                    start=(j == 0), stop=(j == NT - 1))
                            for ch in range(2):
                                cs = slice(ch * 512, (ch + 1) * 512)
                                nc.tensor.matmul(
                                    o_ps[:, cs],
                                    lhsT=VT[:, j, h * P:(h + 1) * P],
                                    rhs=ex[:, cs],
                                    start=(j == 0), stop=(j == NT - 1))
                        zs = ap.tile([1, K], f32, tag="zs")
                        nc.vector.reciprocal(out=zs[:], in_=z_ps[:])
                        zb = ap.tile([P, K], f32, tag="zb")
                        nc.gpsimd.partition_broadcast(zb[:], zs[:])
                        nc.vector.tensor_mul(out=ATT[:, h, :], in0=o_ps[:],
                                             in1=zb[:])

                if upto <= 4:
                    return
                # wo + transpose token-major + pairwise AllReduce
                with tc.tile_pool(name="wo2", bufs=3) as wp2, \
                     tc.tile_pool(name="atj", bufs=1) as atjp, \
                     tc.tile_pool(name="wps", bufs=2, space="PSUM") as wps, \
                     tc.tile_pool(name="tps", bufs=4, space="PSUM") as tps:
                    ATJ = [atjp.tile([P, D], bf16, name=f"atj{j}") for j in range(NT)]
                    for c2 in range(ND):
                        wos = wp2.tile([P, NT, P], bf16, tag="wos")
                        nc.sync.dma_start(
                            out=wos[:],
                            in_=wot[c2].rearrange("(t p) n -> p t n", p=P))
                        ao_ps = wps.tile([P, K], f32, space="PSUM", tag="aops")
                        for tt in range(NT):
                            for ch in range(2):
                                cs = slice(ch * 512, (ch + 1) * 512)
                                nc.tensor.matmul(
                                    ao_ps[:, cs], lhsT=wos[:, tt, :],
                                    rhs=ATT[:, tt, cs],
                                    start=(tt == 0), stop=(tt == NT - 1))
                        aos = wp2.tile([P, K], bf16, tag="aos")
                        nc.scalar.copy(out=aos[:], in_=ao_ps[:])
                        for j in range(NT):
                            tp = tps.tile([P, P], bf16, space="PSUM", tag="ttp")
                            nc.tensor.transpose(
                                out=tp[:], in_=aos[:, j * P:(j + 1) * P],
                                identity=ident_b[:])
                            nc.scalar.copy(out=ATJ[j][:, c2 * P:(c2 + 1) * P],
                                           in_=tp[:])
                    for j in range(NT):
                        nc.sync.dma_start(out=atd[j * P:(j + 1) * P, :],
                                          in_=ATJ[j][:])
        if solo:
            nc.sync.dma_start(out=atrd[:], in_=atd[:])
        else:
            nc.gpsimd.collective_compute(
                "AllReduce", OP.add, replica_groups=GROUPS,
                ins=[atd[:]], outs=[atrd[:]])

        if upto <= 5:
            return
        # ============ x1 = fx + attn; rmsnorm2; h2 feature-major ============
        with tc.tile_pool(name="h2p", bufs=1) as h2p:
            H2F = h2p.tile([P, ND, K], bf16)
            with tc.tile_pool(name="res", bufs=3) as rp2, \
                 tc.tile_pool(name="rps2", bufs=4, space="PSUM") as rps2:
                for j in range(NT):
                    fxt = rp2.tile([P, D], f32, tag="fx2")
                    nc.gpsimd.indirect_dma_start(
                        out=fxt[:], out_offset=None, in_=xr[:],
                        in_offset=bass.IndirectOffsetOnAxis(ap=SI[:, j:j + 1],
                                                            axis=0))
                    at = rp2.tile([P, D], bf16, tag="at")
                    nc.sync.dma_start(out=at[:], in_=atrd[j * P:(j + 1) * P, :])
                    x1 = rp2.tile([P, D], f32, tag="x1")
                    nc.vector.tensor_add(out=x1[:], in0=fxt[:], in1=at[:])
                    # ybase = fx + rw*x1; the tail then only adds rw*ffn
                    yb = rp2.tile([P, D], f32, tag="yb")
                    nc.vector.scalar_tensor_tensor(
                        out=yb[:], in0=x1[:], scalar=RW[:, j:j + 1],
                        in1=fxt[:], op0=OP.mult, op1=OP.add)
                    nc.sync.dma_start(out=x1d[j * P:(j + 1) * P, :], in_=yb[:])
                    h2 = rp2.tile([P, D], bf16, tag="h2t")
                    _rmsnorm_tile(nc, rp2, x1, h2, "r")
                    for i in range(ND):
                        tp = rps2.tile([P, P], bf16, space="PSUM", tag="tp2")
                        nc.tensor.transpose(out=tp[:],
                                            in_=h2[:, i * P:(i + 1) * P],
                                            identity=ident_b[:])
                        nc.scalar.copy(out=H2F[:, i, j * P:(j + 1) * P],
                                       in_=tp[:])

            if upto <= 6:
                return
            # ================= phase F: FFN =================
            with tc.tile_pool(name="gbuf", bufs=1) as gbp:
                GT = gbp.tile([P, NF, K], bf16)
                with tc.tile_pool(name="fw", bufs=3) as fw, \
                     tc.tile_pool(name="fps", bufs=2, space="PSUM") as fps:
                    for c in range(NF):
                        w1s = fw.tile([P, ND, P], bf16, tag="w1s")
                        nc.sync.dma_start(
                            out=w1s[:],
                            in_=w1t[c].rearrange("(i p) n -> p i n", p=P))
                        u_ps = fps.tile([P, K], f32, space="PSUM", tag="ups")
                        for i in range(ND):
                            for ch in range(2):
                                cs = slice(ch * 512, (ch + 1) * 512)
                                nc.tensor.matmul(
                                    u_ps[:, cs], lhsT=w1s[:, i, :],
                                    rhs=H2F[:, i, cs],
                                    start=(i == 0), stop=(i == ND - 1))
                        su = fw.tile([P, K], bf16, tag="su")
                        nc.scalar.activation(out=su[:], in_=u_ps[:],
                                             func=ACT.Silu)
                        w3s = fw.tile([P, ND, P], bf16, tag="w3s")
                        nc.sync.dma_start(
                            out=w3s[:],
                            in_=w3t[c].rearrange("(i p) n -> p i n", p=P))
                        v_ps = fps.tile([P, K], f32, space="PSUM", tag="vps2")
                        for i in range(ND):
                            for ch in range(2):
                                cs = slice(ch * 512, (ch + 1) * 512)
                                nc.tensor.matmul(
                                    v_ps[:, cs], lhsT=w3s[:, i, :],
                                    rhs=H2F[:, i, cs],
                                    start=(i == 0), stop=(i == ND - 1))
                        nc.vector.tensor_mul(out=GT[:, c, :], in0=su[:],
                                             in1=v_ps[:])

                if upto <= 7:
                    return
                with tc.tile_pool(name="f2", bufs=2) as f2, \
                     tc.tile_pool(name="ftj", bufs=1) as ftjp, \
                     tc.tile_pool(name="f2ps", bufs=2, space="PSUM") as f2ps, \
                     tc.tile_pool(name="t2ps", bufs=4, space="PSUM") as t2ps:
                    FTJ = [ftjp.tile([P, D], bf16, name=f"ftj{j}") for j in range(NT)]
                    for c2 in range(ND):
                        w2s = f2.tile([P, NF, P], bf16, tag="w2s")
                        nc.sync.dma_start(
                            out=w2s[:],
                            in_=w2t[c2].rearrange("(c p) n -> p c n", p=P))
                        f_ps = f2ps.tile([P, K], f32, space="PSUM", tag="fps2")
                        for c in range(NF):
                            for ch in range(2):
                                cs = slice(ch * 512, (ch + 1) * 512)
                                nc.tensor.matmul(
                                    f_ps[:, cs], lhsT=w2s[:, c, :],
                                    rhs=GT[:, c, cs],
                                    start=(c == 0), stop=(c == NF - 1))
                        fos = f2.tile([P, K], bf16, tag="fos")
                        nc.scalar.copy(out=fos[:], in_=f_ps[:])
                        for j in range(NT):
                            tp = t2ps.tile([P, P], bf16, space="PSUM", tag="t2p")
                            nc.tensor.transpose(
                                out=tp[:], in_=fos[:, j * P:(j + 1) * P],
                                identity=ident_b[:])
                            nc.scalar.copy(out=FTJ[j][:, c2 * P:(c2 + 1) * P],
                                           in_=tp[:])
                    for j in range(NT):
                        nc.sync.dma_start(out=ftd[j * P:(j + 1) * P, :],
                                          in_=FTJ[j][:])
        if solo:
            nc.sync.dma_start(out=ftrd[:], in_=ftd[:])
        else:
            nc.gpsimd.collective_compute(
                "AllReduce", OP.add, replica_groups=GROUPS,
                ins=[ftd[:]], outs=[ftrd[:]])

        if upto <= 8:
            return
        # ============ tail: x2 = x1 + ffn; y = fx + rw*x2; scatter ============
        with tc.tile_pool(name="tail", bufs=4) as tl:
            for j in range(NT):
                yb = tl.tile([P, D], f32, tag="ybt")
                nc.sync.dma_start(out=yb[:], in_=x1d[j * P:(j + 1) * P, :])
                ft = tl.tile([P, D], bf16, tag="ftt")
                nc.sync.dma_start(out=ft[:], in_=ftrd[j * P:(j + 1) * P, :])
                y = tl.tile([P, D], f32, tag="yt")
                nc.vector.scalar_tensor_tensor(
                    out=y[:], in0=ft[:], scalar=RW[:, j:j + 1], in1=yb[:],
                    op0=OP.mult, op1=OP.add)
                nc.gpsimd.indirect_dma_start(
                    out=outr[:], out_offset=bass.IndirectOffsetOnAxis(
                        ap=SIDX[:, j:j + 1], axis=0),
                    in_=y[:], in_offset=None,
                    bounds_check=HALF - 1, oob_is_err=False)


_NC_CACHE = None


def _get_program():
    global _NC_CACHE
    if _NC_CACHE is None:
        _NC_CACHE = _build_program()
    return _NC_CACHE


def make_in_maps(x, mask, freqs_cis, w_router, g1, wq, wk, wv, wo, g2,
                 w1, w3, w2):
    # g1/g2 are ones and mask is zeros per the problem spec; both are folded
    # out of the device program.
    import ml_dtypes
    bf = ml_dtypes.bfloat16

    x = np.asarray(x, np.float32)
    cos = np.ascontiguousarray(np.asarray(freqs_cis)[..., 0].T, dtype=np.float32)
    sin = np.ascontiguousarray(np.asarray(freqs_cis)[..., 1].T, dtype=np.float32)
    wrb = np.ascontiguousarray(np.broadcast_to(
        np.asarray(w_router, np.float32), (P, D)))
    iota = np.arange(P, dtype=np.float32).reshape(P, 1)
    itok = (np.arange(P)[:, None] * 32 + np.arange(32)[None, :]).astype(np.int32)

    wq, wk, wv = (np.asarray(w, np.float32) for w in (wq, wk, wv))
    wo, w1, w3, w2 = (np.asarray(w, np.float32) for w in (wo, w1, w3, w2))

    def tile3(w, n):
        return np.ascontiguousarray(
            w.reshape(w.shape[0], n, P).transpose(1, 0, 2)).astype(bf)

    in_maps = []
    for c in range(NCORE):
        p, h = c // 2, c % 2
        in_maps.append({
            "xr": x[p],
            "xregion": np.ascontiguousarray(x[p, h * HALF:(h + 1) * HALF]),
            "selbase": np.full((P, 1), float(h * HALF), np.float32),
            "wrb": wrb,
            "cosT": cos, "sinT": sin,
            "iotaP": iota, "iotaP1": iota + 1.0,
            "itok": itok,
            "wqt": tile3(wq[:, h * DH:(h + 1) * DH], HEADS),
            "wkt": tile3(wk[:, h * DH:(h + 1) * DH], HEADS),
            "wvt": wv[:, h * DH:(h + 1) * DH].astype(bf),
            "wot": tile3(wo[h * DH:(h + 1) * DH, :], ND),
            "w1t": tile3(w1[:, h * FH:(h + 1) * FH], NF),
            "w3t": tile3(w3[:, h * FH:(h + 1) * FH], NF),
            "w2t": tile3(w2[h * FH:(h + 1) * FH, :], ND),
        })
    return in_maps


def assemble_output(results):
    out = np.empty((B, S, D), np.float32)
    for c in range(NCORE):
        p, h = c // 2, c % 2
        out[p, h * HALF:(h + 1) * HALF] = results[c]["outr"]
    return out


def kernel(**inputs) -> np.ndarray:
    nc = _get_program()
    in_maps = make_in_maps(**inputs)
    res = run_bass_kernel_spmd(nc, in_maps, list(range(NCORE)))
    return assemble_output(res.results)

